# revision 32
# baseline (speedup 1.0000x reference)
"""Trainium2 Bass kernel for 2-layer GAT (nn_GAT_66821101191576).

v5: fp8(e4m3) layer-2 gather table (512B rows vs 768B), per-core local
projection tables (duplicated A-phase compute), contiguous
p-major table writes, batched ximg input layout, batched G-phase elementwise
ops, blocking chained gathers.

Self-contained: hardcodes shapes, does host-side graph preprocessing,
builds/compiles a Tile/Bass SPMD program for 8 NeuronCores, runs via
run_bass_kernel_spmd, and reassembles the full output.
"""

import contextlib
import ctypes
import sys
import types

for _p in ("/opt/trn_rl_repo", "/opt/pypackages"):
    if _p not in sys.path:
        sys.path.insert(0, _p)

import numpy as np

import concourse.bacc as bacc
import concourse.bass as bass
import concourse.mybir as mybir
import concourse.tile as tile
from concourse.bass_utils import run_bass_kernel_spmd

F32 = mybir.dt.float32
BF16 = mybir.dt.bfloat16
FP8 = mybir.dt.float8e4
I16 = mybir.dt.int16
AF = mybir.ActivationFunctionType
OP = mybir.AluOpType

# ---------------------------------------------------------------- problem dims
N = 20000
E = 640000
IN = 256
H = 8
HF = 64
OUT = 32
NEG_SLOPE = 0.2

NCORES = 8
NEG_BIG = -1.0e30

PN = 2560              # padded nodes per core
CH = 20                # G-phase chunks per core (128 dst each)
NSLOT = NCORES * PN    # 20480
PADROW = NSLOT         # the single pad row every padding edge points at
NROWS = NSLOT + 128
D1 = H * HF            # 512
D2 = H * OUT           # 256
RW1 = D1 + 128         # 640 bf16 cols = 1280B rows (gather rows: 256B mult)
RW2 = 512              # fp8 rows: 256 feat fp8 + 32B att bf16 + pad = 512B

# groups within a core's PN rows: 1024,1024,512 rows -> 8,8,4 sub-rows/part
GJ = (8, 8, 4)
GBASE = (0, 1024, 2048)

LAST_EXEC_TIME_NS = None


def _install_ntff_hook():
    """Provide antenv.axon_hooks (NTFF profiling) if the image lacks it."""
    try:
        from antenv.axon_hooks import get_axon_ntff_profile_hook  # noqa: F401
        return
    except ImportError:
        pass
    so_path = "/opt/axon/libaxon_pjrt.so"
    try:
        lib = ctypes.CDLL(so_path)
    except OSError:
        return
    if not hasattr(lib, "axon_start_nrt_profile"):
        return
    lib.axon_start_nrt_profile.argtypes = [
        ctypes.POINTER(ctypes.c_int64),
        ctypes.c_size_t,
    ]
    lib.axon_start_nrt_profile.restype = ctypes.c_int64
    lib.axon_stop_nrt_profile.argtypes = [ctypes.c_char_p]
    lib.axon_stop_nrt_profile.restype = ctypes.c_int64

    @contextlib.contextmanager
    def _hook(output_dir, device_ids):
        import jax

        jax.devices()
        if device_ids:
            ids = (ctypes.c_int64 * len(device_ids))(*device_ids)
            rc = lib.axon_start_nrt_profile(ids, len(device_ids))
        else:
            rc = lib.axon_start_nrt_profile(None, 0)
        if rc != 0:
            raise RuntimeError(f"axon_start_nrt_profile rc={rc}")
        try:
            yield
        finally:
            n = lib.axon_stop_nrt_profile(str(output_dir).encode())
            print(f"ntff profile: {n} file(s) -> {output_dir}", file=sys.stderr)

    mod = types.ModuleType("antenv.axon_hooks")
    mod.get_axon_ntff_profile_hook = lambda: _hook
    mod.set_axon_ntff_profile_hook = lambda h: None
    sys.modules["antenv.axon_hooks"] = mod
    import antenv

    antenv.axon_hooks = mod


# ------------------------------------------------------------ host preprocessing
def _chunk_part_to_local(c, p):
    """local slot for (G-phase chunk c, partition p) within a core (vectorized)."""
    c = np.asarray(c)
    g = c // 8
    j = c - g * 8
    gbase = np.asarray(GBASE)[g]
    gj = np.asarray(GJ)[g]
    return gbase + p * gj + j


def _local_to_chunk_part(l):
    """(chunk, partition) for a local slot (vectorized)."""
    g = np.minimum(l // 1024, 2)
    rem = l - g * 1024
    jg = np.where(g < 2, 8, 4)
    p = rem // jg
    j = rem - p * jg
    return g * 8 + j, p


def _wrap_idx16(flat):
    """Wrap a flat int16 index vector into the dma_gather SBUF image:
    index i -> partition i%16, column i//16, replicated 8x down partitions."""
    assert flat.size % 16 == 0
    v = flat.reshape(-1, 16).T.astype(np.int16)
    return np.tile(v, (8, 1))


def preprocess(x, edge_index, W1, att_src1, att_dst1, b1, W2, att_src2, att_dst2, b2):
    src = np.asarray(edge_index[0], dtype=np.int64)
    dst = np.asarray(edge_index[1], dtype=np.int64)
    loops = np.arange(N, dtype=np.int64)
    src = np.concatenate([src, loops])
    dst = np.concatenate([dst, loops])

    deg = np.bincount(dst, minlength=N)
    order = np.argsort(-deg, kind="stable")  # rank -> node
    ranks = np.arange(N)
    # rank r -> core r%8, chunk (r//8)//128, partition (r//8)%128
    core = ranks % NCORES
    cc = (ranks // NCORES) // 128
    pp = (ranks // NCORES) % 128
    node2slot = np.empty(N, dtype=np.int64)
    node2slot[order] = core * PN + _chunk_part_to_local(cc, pp)
    # rank -> output row (out_d is chunk-major: c*128+p), per core
    node2outrow = np.empty(N, dtype=np.int64)
    node2outrow[order] = core * PN + cc * 128 + pp

    es = node2slot[src]
    ed = node2slot[dst]

    # per-destination edge schedule
    o2 = np.argsort(ed, kind="stable")
    s_src = es[o2]
    s_dst = ed[o2]
    starts = np.searchsorted(s_dst, np.arange(NSLOT))
    kwithin = np.arange(s_dst.size) - starts[s_dst]

    deg_slot = np.bincount(ed, minlength=NSLOT)
    dloc = deg_slot.reshape(NCORES, PN)
    lc, lp = _local_to_chunk_part(np.arange(PN))
    dmat = np.zeros((NCORES, CH, 128), dtype=np.int64)
    dmat[:, lc, lp] = dloc
    dsched = dmat.max(axis=(0, 2))
    dsched = np.maximum(((dsched + 7) // 8) * 8, 8).astype(np.int64)
    choff = np.concatenate([[0], np.cumsum(dsched)])
    TOTD = int(choff[-1])

    # per-core [TOTD, 128] neighbor tables filled with PADROW
    arr = np.full((NCORES, TOTD, 128), PADROW, dtype=np.int64)
    e_core = s_dst // PN
    e_loc = s_dst % PN
    e_ch, e_p = _local_to_chunk_part(e_loc)
    arr[e_core, choff[e_ch] + kwithin, e_p] = s_src
    assert arr.max() <= 32767

    idx_imgs = [_wrap_idx16(arr[k].reshape(-1).astype(np.int16)) for k in range(NCORES)]
    own_imgs = []
    ownrows = np.empty(CH * 128, dtype=np.int64)
    for c in range(CH):
        ownrows[c * 128 : (c + 1) * 128] = _chunk_part_to_local(c, np.arange(128))
    for k in range(NCORES):
        own_imgs.append(_wrap_idx16((k * PN + ownrows).astype(np.int16)))

    # permuted node features (by slot), transposed, padded
    xT = np.zeros((IN, NSLOT), dtype=np.float32)
    xT[:, node2slot] = np.asarray(x, dtype=np.float32).T

    # W extensions: columns reordered (c*H + h); attention projections folded in
    W1r = np.asarray(W1, np.float32).reshape(IN, H, HF)
    w1p = W1r.transpose(0, 2, 1).reshape(IN, H * HF)
    a1s = np.einsum("ihc,hc->ih", W1r, np.asarray(att_src1, np.float32))
    a1d = np.einsum("ihc,hc->ih", W1r, np.asarray(att_dst1, np.float32))
    w1e = np.concatenate([w1p, a1s, a1d], axis=1).astype(np.float32)  # [IN, 528]

    W2r = np.asarray(W2, np.float32).reshape(HF, H, OUT)
    w2p = W2r.transpose(0, 2, 1).reshape(HF, H * OUT)
    a2s = np.einsum("ihc,hc->ih", W2r, np.asarray(att_src2, np.float32))
    a2d = np.einsum("ihc,hc->ih", W2r, np.asarray(att_dst2, np.float32))
    w2e = np.concatenate([w2p, a2s, a2d], axis=1).astype(np.float32)  # [HF, 272]

    import ml_dtypes

    b1b = np.tile(np.asarray(b1, np.float32)[None, :], (128, 1))
    b2b = np.tile(np.asarray(b2, np.float32)[None, :], (128, 1))
    identf = np.eye(128, dtype=np.float32)
    identbf = np.eye(128, dtype=ml_dtypes.bfloat16)

    # ximg per parity: A1 lhsT tiles for core-blocks [par, par+2, par+4, par+6]
    # laid out per A-chunk a = i*CH + c:
    # ximg[p, a*256 + kk*128 + n] = x-feat (kk*128+p) of dst-node (chunk c, part n)
    xT_b = xT.astype(ml_dtypes.bfloat16)
    im = np.zeros((128, 160 * 256), dtype=ml_dtypes.bfloat16)
    a = 0
    for kb in range(NCORES):
        for c in range(CH):
            slots = kb * PN + ownrows[c * 128 : (c + 1) * 128]
            blk = xT_b[:, slots]  # [256, 128]
            im[:, a * 256 : a * 256 + 128] = blk[0:128]
            im[:, a * 256 + 128 : a * 256 + 256] = blk[128:256]
            a += 1
    ximg = im

    shared = {
        "w1e": w1e.astype(ml_dtypes.bfloat16),
        "w2e": w2e.astype(ml_dtypes.bfloat16),
        "b1b": b1b,
        "b2b": b2b,
        "identf": identf,
        "identbf": identbf,
    }
    in_maps = []
    for k in range(NCORES):
        m = dict(shared)
        m["ximg"] = ximg
        m["idxs"] = idx_imgs[k]
        m["ownidx"] = own_imgs[k]
        in_maps.append(m)

    meta = {
        "dsched": tuple(int(d) for d in dsched),
        "choff": tuple(int(c) for c in choff),
        "node2outrow": node2outrow,
    }
    return in_maps, meta


# ------------------------------------------------------------------ the program
def build_program(meta, ncores=NCORES, enable_asserts=False, debug=False):
    dsched = meta["dsched"]
    choff = meta["choff"]
    TOTD = choff[-1]

    nc = bacc.Bacc(
        "TRN2",
        target_bir_lowering=False,
        debug=debug,
        enable_asserts=enable_asserts,
        num_devices=ncores,
        num_swdge_queues=4,
    )

    # ---- I/O
    ximg_d = nc.dram_tensor("ximg", [128, 160 * 256], BF16, kind="ExternalInput")
    w1e = nc.dram_tensor("w1e", [IN, D1 + 16], BF16, kind="ExternalInput")
    w2e = nc.dram_tensor("w2e", [HF, D2 + 16], BF16, kind="ExternalInput")
    b1b_d = nc.dram_tensor("b1b", [128, HF], F32, kind="ExternalInput")
    b2b_d = nc.dram_tensor("b2b", [128, OUT], F32, kind="ExternalInput")
    identf_d = nc.dram_tensor("identf", [128, 128], F32, kind="ExternalInput")
    identbf_d = nc.dram_tensor("identbf", [128, 128], BF16, kind="ExternalInput")
    idxs_d = nc.dram_tensor("idxs", [128, TOTD * 8], I16, kind="ExternalInput")
    ownidx_d = nc.dram_tensor("ownidx", [128, PN // 16], I16, kind="ExternalInput")
    out_d = nc.dram_tensor("out", [PN, OUT], F32, kind="ExternalOutput")

    # ---- pair-shared tables (one physical copy per 2-core HBM domain)
    h1_tbl = nc.dram_tensor("h1_tbl", [NROWS, RW1], BF16)
    h2_tbl = nc.dram_tensor("h2_tbl", [NROWS, RW2], FP8)
    h1t_loc = nc.dram_tensor("h1t_loc", [HF, PN], BF16)
    h1t_all = nc.dram_tensor("h1t_all", [ncores, HF, PN], BF16, addr_space="Shared")

    _swprev = [None]
    _swq = [0]

    def _gather(out_ap, in_ap, idxs_ap, nidx, elem):
        q = _swq[0] % 4
        _swq[0] += 1
        inst = nc.gpsimd.dma_gather(
            out_ap, in_ap, idxs_ap, nidx, nidx, elem, queue_num=q,
            single_packet=False,
        )
        if _swprev[0] is not None:
            bass._add_dep_helper(
                inst.ins, _swprev[0].ins, sync=False, reason="swdge order"
            )
        _swprev[0] = inst
        return inst

    with tile.TileContext(nc) as tc:

        with contextlib.ExitStack() as big:
            cpool = big.enter_context(tc.tile_pool(name="consts", bufs=1))
            w1_sb = cpool.tile([128, 2, D1 + 16], BF16)
            nc.sync.dma_start(w1_sb[:], w1e[:, :].rearrange("(k p) c -> p k c", p=128))
            w2_sb = cpool.tile([HF, D2 + 16], BF16)
            nc.sync.dma_start(w2_sb[:], w2e[:, :])
            b1_sb = cpool.tile([128, HF], F32)
            nc.sync.dma_start(b1_sb[:], b1b_d[:, :])
            b2_sb = cpool.tile([128, OUT], F32)
            nc.sync.dma_start(b2_sb[:], b2b_d[:, :])
            idf_sb = cpool.tile([128, 128], F32)
            nc.sync.dma_start(idf_sb[:], identf_d[:, :])
            idb_sb = cpool.tile([128, 128], BF16)
            nc.sync.dma_start(idb_sb[:], identbf_d[:, :])
            idx_sb = cpool.tile([128, TOTD * 8], I16)
            nc.sync.dma_start(idx_sb[:], idxs_d[:, :])
            own_sb = cpool.tile([128, PN // 16], I16)
            nc.sync.dma_start(own_sb[:], ownidx_d[:, :])
            neg_sb = cpool.tile([1, 8], BF16)
            nc.vector.memset(neg_sb[:], NEG_BIG)
            zro_sb = cpool.tile([1, D1], BF16)
            nc.vector.memset(zro_sb[:], 0.0)
            zro8_sb = cpool.tile([1, D2], FP8)
            nc.vector.memset(zro8_sb[:], 0.0)
            eps_sb = cpool.tile([128, 1], F32)
            nc.vector.memset(eps_sb[:], 1e-30)
            h1t_sb = cpool.tile([HF, PN], BF16)

            # ================ phase A helper: project all 8 core-blocks;
            # rows blk*PN + GBASE[g] + p*jg + j, written contiguously.
            def phase_A(lhsT_of, w_of, kparts, dcols, rw, tbl, tag, fp8=False):
                fuse = dcols + 16 <= 512
                last_write = [None]
                with contextlib.ExitStack() as st:
                    lp = st.enter_context(tc.tile_pool(name=f"a{tag}l", bufs=3))
                    pp = st.enter_context(
                        tc.tile_pool(name=f"a{tag}p", bufs=3, space="PSUM")
                    )
                    sp = st.enter_context(tc.tile_pool(name=f"a{tag}s", bufs=4))
                    a = 0
                    for blk in range(ncores):
                        for g in range(3):
                            jg = GJ[g]
                            hst = sp.tile(
                                [128, 8, rw], FP8 if fp8 else BF16, tag=f"hst{tag}"
                            )
                            for j in range(jg):
                                lhsT = lhsT_of(lp, a)
                                a += 1
                                if fuse:
                                    ps = pp.tile([128, dcols + 16], F32, tag=f"ps{tag}")
                                    for kk in range(kparts):
                                        nc.tensor.matmul(
                                            ps[:],
                                            lhsT(kk),
                                            w_of(kk)[:, 0 : dcols + 16],
                                            start=(kk == 0),
                                            stop=(kk == kparts - 1),
                                        )
                                    if fp8:
                                        if j % 2 == 0:
                                            nc.scalar.copy(
                                                hst[:, j, 0:dcols], ps[:, 0:dcols]
                                            )
                                        else:
                                            nc.vector.tensor_copy(
                                                hst[:, j, 0:dcols], ps[:, 0:dcols]
                                            )
                                        nc.vector.tensor_copy(
                                            hst[:, j, dcols : dcols + 32].bitcast(BF16),
                                            ps[:, dcols : dcols + 16],
                                        )
                                    elif j % 2 == 0:
                                        nc.scalar.copy(hst[:, j, 0 : dcols + 16], ps[:])
                                    else:
                                        nc.vector.tensor_copy(
                                            hst[:, j, 0 : dcols + 16], ps[:]
                                        )
                                else:
                                    ps = pp.tile([128, dcols], F32, tag=f"ps{tag}")
                                    ps2 = pp.tile([128, 16], F32, tag=f"ps2{tag}")
                                    for kk in range(kparts):
                                        nc.tensor.matmul(
                                            ps[:],
                                            lhsT(kk),
                                            w_of(kk)[:, 0:dcols],
                                            start=(kk == 0),
                                            stop=(kk == kparts - 1),
                                        )
                                        nc.tensor.matmul(
                                            ps2[:],
                                            lhsT(kk),
                                            w_of(kk)[:, dcols : dcols + 16],
                                            start=(kk == 0),
                                            stop=(kk == kparts - 1),
                                        )
                                    if j % 2 == 0:
                                        nc.scalar.copy(hst[:, j, 0:dcols], ps[:])
                                    else:
                                        nc.vector.tensor_copy(hst[:, j, 0:dcols], ps[:])
                                    nc.vector.tensor_copy(
                                        hst[:, j, dcols : dcols + 16], ps2[:]
                                    )
                            row0 = blk * PN + GBASE[g]
                            w = nc.sync.dma_start(
                                tbl[row0 : row0 + jg * 128, :].rearrange(
                                    "(p j) c -> p j c", p=128
                                ),
                                hst[:, 0:jg, :],
                            )
                            last_write[0] = w
                return last_write[0]

            # ---------------- A1 (x tiles loaded one core-block at a time)
            _xblk = [None, -1]

            def lhsT_of_A1(lp, a):
                blk = a // CH
                if _xblk[1] != blk:
                    xt = lp.tile([128, CH * 256], BF16, tag="xt")
                    nc.sync.dma_start(
                        xt[:], ximg_d[:, blk * CH * 256 : (blk + 1) * CH * 256]
                    )
                    _xblk[0] = xt
                    _xblk[1] = blk
                base = (a % CH) * 256
                xt = _xblk[0]
                return lambda kk: xt[:, base + kk * 128 : base + (kk + 1) * 128]

            lastw1 = phase_A(
                lhsT_of_A1, lambda kk: w1_sb[:, kk, :], 2, D1, RW1, h1_tbl, "1"
            )

            # PADROW fixups (both cores of a pair write the same values)
            f1 = nc.sync.dma_start(h1_tbl[PADROW : PADROW + 1, 0:D1], zro_sb[:, 0:D1])
            f2 = nc.sync.dma_start(
                h1_tbl[PADROW : PADROW + 1, D1 : D1 + 8], neg_sb[:, 0:8]
            )


            # ================ gather/aggregate phase
            def phase_G(tbl, dcols, rw, chans, b_sb, relu, sink, tag, hbufs,
                        fp8=False, after_chunk=None):
                with contextlib.ExitStack() as st:
                    gp = st.enter_context(tc.tile_pool(name=f"g{tag}g", bufs=1))
                    hp = st.enter_context(tc.tile_pool(name=f"g{tag}h", bufs=hbufs))
                    sp = st.enter_context(tc.tile_pool(name=f"g{tag}s", bufs=3))
                    mp = st.enter_context(tc.tile_pool(name=f"g{tag}m", bufs=4))
                    xp = st.enter_context(tc.tile_pool(name=f"g{tag}x", bufs=2))
                    pp = st.enter_context(
                        tc.tile_pool(name=f"g{tag}p", bufs=2, space="PSUM")
                    )
                    def gather(out_ap, idxs_ap, nidx, need_bar):
                        _gather(out_ap, tbl[:, :], idxs_ap, nidx, rw)

                    # own rows (adst at dcols+8:dcols+16): self-written in
                    # phase A (parity blocks), so no barrier needed.
                    atown = gp.tile(
                        [128, CH, rw], FP8 if fp8 else BF16, tag=f"at{tag}"
                    )
                    off = 0
                    while off < PN:
                        nn = min(1024, PN - off)
                        gather(
                            atown[:, off // 128 : (off + nn) // 128, :],
                            own_sb[:, off // 16 : (off + nn) // 16],
                            nn,
                            False,
                        )
                        off += nn

                    for c in range(CH):
                        D = dsched[c]
                        col0 = choff[c] * 8
                        ex = mp.tile([128, 64, H], BF16, tag=f"ex{tag}")
                        ps = pp.tile([128, dcols], F32, tag=f"ps{tag}")
                        if fp8:
                            adst_c = atown[:, c, dcols + 16 : dcols + 32].bitcast(BF16)
                        else:
                            adst_c = atown[:, c, dcols + 8 : dcols + 16]
                        first = True
                        for d0 in range(0, D, 16):
                            dn = min(16, D - d0)
                            hg = hp.tile(
                                [128, 16, rw], FP8 if fp8 else BF16, tag=f"hg{tag}"
                            )
                            for dd in range(0, dn, 8):
                                gather(
                                    hg[:, dd : dd + 8, :],
                                    idx_sb[
                                        :,
                                        col0 + (d0 + dd) * 8 : col0 + (d0 + dd + 8) * 8,
                                    ],
                                    1024,
                                    True,
                                )
                            # logits for the whole 32-slice group
                            epre = mp.tile([128, 16, H], F32, tag=f"ep{tag}")
                            adst_b = adst_c.unsqueeze(1).broadcast_to([128, dn, H])
                            asrc_v = (
                                hg[:, 0:dn, dcols : dcols + 16].bitcast(BF16)
                                if fp8
                                else hg[:, 0:dn, dcols : dcols + 8]
                            )
                            nc.vector.tensor_tensor(
                                epre[:, 0:dn, :], asrc_v, adst_b, OP.add
                            )
                            e = mp.tile([128, 16, H], F32, tag=f"e{tag}")
                            nc.vector.scalar_tensor_tensor(
                                e[:, 0:dn, :],
                                epre[:, 0:dn, :],
                                NEG_SLOPE,
                                epre[:, 0:dn, :],
                                OP.mult,
                                OP.max,
                            )
                            nc.scalar.activation(
                                ex[:, d0 : d0 + dn, :], e[:, 0:dn, :], AF.Exp
                            )
                            for dd in range(0, dn, 8):
                                sg = sp.tile([128, 8, dcols], BF16, tag=f"sg{tag}")
                                hg_v = hg[:, dd : dd + 8, 0:dcols].rearrange(
                                    "p d (c h) -> p d c h", h=H
                                )
                                sg_v = sg[:].rearrange("p d (c h) -> p d c h", h=H)
                                ex_v = (
                                    ex[:, d0 + dd : d0 + dd + 8, :]
                                    .unsqueeze(2)
                                    .broadcast_to([128, 8, chans, H])
                                )
                                nc.vector.tensor_tensor(sg_v, hg_v, ex_v, OP.mult)
                                for j in range(8):
                                    nc.tensor.matmul(
                                        ps[:],
                                        idb_sb[:],
                                        sg[:, j, :],
                                        start=first,
                                        stop=(d0 + dd + 8 >= D and j == 7),
                                    )
                                    first = False
                        den = mp.tile([128, H], F32, tag=f"den{tag}")
                        nc.vector.reduce_sum(
                            den[:],
                            ex[:, 0:D, :].transpose([0, 2, 1]),
                            axis=mybir.AxisListType.X,
                        )
                        nc.scalar.activation(
                            den[:], den[:], AF.Identity, bias=eps_sb[:, 0:1]
                        )
                        rden = mp.tile([128, H], F32, tag=f"rd{tag}")
                        nc.vector.reciprocal(rden[:], den[:])
                        acc = mp.tile([128, dcols], F32, tag=f"ac{tag}")
                        nc.scalar.copy(acc[:], ps[:])
                        t1 = mp.tile([128, dcols], F32, tag=f"t1{tag}")
                        rden_b = rden[:].unsqueeze(1).broadcast_to([128, chans, H])
                        nc.vector.tensor_tensor(
                            t1[:].rearrange("p (c h) -> p c h", h=H),
                            acc[:].rearrange("p (c h) -> p c h", h=H),
                            rden_b,
                            OP.mult,
                        )
                        hsum = mp.tile([128, chans], F32, tag=f"hs{tag}")
                        nc.vector.reduce_sum(
                            hsum[:],
                            t1[:].rearrange("p (c h) -> p c h", h=H),
                            axis=mybir.AxisListType.X,
                        )
                        res = mp.tile([128, chans], F32, tag=f"rs{tag}")
                        nc.vector.scalar_tensor_tensor(
                            res[:], hsum[:], 1.0 / H, b_sb[:], OP.mult, OP.add
                        )
                        if relu:
                            resf = mp.tile([128, chans], F32, tag=f"rf{tag}")
                            nc.scalar.activation(resf[:], res[:], AF.Relu)
                        else:
                            resf = res
                        sink(c, resf, xp, pp)
                        if after_chunk is not None:
                            after_chunk(c)

            # ---------------- G1 -> h1t_sb, AG + A2 overlapped in halves
            def sink1(c, resf, xp, pp):
                tps = pp.tile([HF, 128], F32, tag="tps")
                nc.tensor.transpose(tps[:], resf[:], idf_sb[:])
                nc.scalar.copy(h1t_sb[:, c * 128 : (c + 1) * 128], tps[:])

            phase_G(h1_tbl, D1, RW1, HF, b1_sb, True, sink1, "1", 5)
            nc.sync.dma_start(h1t_loc[:, :], h1t_sb[:])
            nc.gpsimd.collective_compute(
                "AllGather",
                OP.bypass,
                replica_groups=[list(range(ncores))],
                ins=[h1t_loc[:, :]],
                outs=[h1t_all[:, :, :]],
            )

            # ---------------- A2: project all nodes from the gathered h1T
            with tc.tile_pool(name="h1tp", bufs=1) as h1tp:
                h1tf = h1tp.tile([HF, NSLOT], BF16)
                nc.sync.dma_start(
                    h1tf[:].rearrange("p (j c) -> p j c", c=PN),
                    h1t_all[:, :, :].transpose([1, 0, 2]),
                )

                def lhsT_of_A2(lp, a):
                    return lambda kk: h1tf[:, a * 128 : (a + 1) * 128]

                lastw2 = phase_A(lhsT_of_A2, lambda kk: w2_sb, 1, D2, RW2, h2_tbl, "2", fp8=True)

            f3 = nc.sync.dma_start(
                h2_tbl[PADROW : PADROW + 1, 0:D2], zro8_sb[:, 0:D2]
            )
            f4 = nc.sync.dma_start(
                h2_tbl[PADROW : PADROW + 1, D2 : D2 + 16].bitcast(BF16),
                neg_sb[:, 0:8],
            )


            # ---------------- G2 -> out
            ostage = [None]

            def sink2(c, resf, xp, pp):
                r = c % 4
                if r == 0:
                    ost = xp.tile([128, 4, OUT], F32, tag="ost")
                    ostage[0] = ost
                nc.vector.tensor_copy(ostage[0][:, r, :], resf[:])
                if r == 3 or c == CH - 1:
                    c0 = c - r
                    nc.sync.dma_start(
                        out_d[c0 * 128 : (c + 1) * 128, :].rearrange(
                            "(r p) c -> p r c", p=128
                        ),
                        ostage[0][:, 0 : r + 1, :],
                    )

            phase_G(h2_tbl, D2, RW2, OUT, b2_sb, False, sink2, "2", 12, fp8=True)

    nc.compile()
    return nc


# ------------------------------------------------------------------ entry point
_CACHE = {}


def _get_program(meta):
    key = (meta["dsched"],)
    if key not in _CACHE:
        _CACHE[key] = build_program(meta)
    return _CACHE[key]


def kernel(x, edge_index, W1, att_src1, att_dst1, b1, W2, att_src2, att_dst2, b2,
           trace=False):
    global LAST_EXEC_TIME_NS
    _install_ntff_hook()
    in_maps, meta = preprocess(
        x, edge_index, W1, att_src1, att_dst1, b1, W2, att_src2, att_dst2, b2
    )
    nc = _get_program(meta)
    res = run_bass_kernel_spmd(nc, in_maps, list(range(NCORES)), trace=trace)
    LAST_EXEC_TIME_NS = res.exec_time_ns
    outs = np.concatenate([res.results[k]["out"] for k in range(NCORES)], axis=0)
    return outs[meta["node2outrow"]].astype(np.float32)


# revision 34
# speedup vs baseline: 1.2326x; 1.2326x over previous
"""Trainium2 Bass kernel for 2-layer GAT (nn_GAT_66821101191576).

v5: fp8(e4m3) layer-2 gather table (512B rows vs 768B), per-core local
projection tables (duplicated A-phase compute), contiguous
p-major table writes, batched ximg input layout, batched G-phase elementwise
ops, blocking chained gathers.

Self-contained: hardcodes shapes, does host-side graph preprocessing,
builds/compiles a Tile/Bass SPMD program for 8 NeuronCores, runs via
run_bass_kernel_spmd, and reassembles the full output.
"""

import contextlib
import ctypes
import sys
import types

for _p in ("/opt/trn_rl_repo", "/opt/pypackages"):
    if _p not in sys.path:
        sys.path.insert(0, _p)

import numpy as np

import concourse.bacc as bacc
import concourse.bass as bass
import concourse.mybir as mybir
import concourse.tile as tile
from concourse.bass_utils import run_bass_kernel_spmd

F32 = mybir.dt.float32
BF16 = mybir.dt.bfloat16
FP8 = mybir.dt.float8e4
I16 = mybir.dt.int16
AF = mybir.ActivationFunctionType
OP = mybir.AluOpType

# ---------------------------------------------------------------- problem dims
N = 20000
E = 640000
IN = 256
H = 8
HF = 64
OUT = 32
NEG_SLOPE = 0.2

NCORES = 8
NEG_BIG = -1.0e30

PN = 2560              # padded nodes per core
CH = 20                # G-phase chunks per core (128 dst each)
NSLOT = NCORES * PN    # 20480
PADROW = NSLOT         # the single pad row every padding edge points at
NROWS = NSLOT + 128
D1 = H * HF            # 512
D2 = H * OUT           # 256
RW1 = D1 + 128         # 640 bf16 cols = 1280B rows (gather rows: 256B mult)
RW2 = 512              # fp8 rows: 256 feat fp8 + 32B att bf16 + pad = 512B

# groups within a core's PN rows: 1024,1024,512 rows -> 8,8,4 sub-rows/part
GJ = (8, 8, 4)
GBASE = (0, 1024, 2048)

LAST_EXEC_TIME_NS = None


def _install_ntff_hook():
    """Provide antenv.axon_hooks (NTFF profiling) if the image lacks it."""
    try:
        from antenv.axon_hooks import get_axon_ntff_profile_hook  # noqa: F401
        return
    except ImportError:
        pass
    so_path = "/opt/axon/libaxon_pjrt.so"
    try:
        lib = ctypes.CDLL(so_path)
    except OSError:
        return
    if not hasattr(lib, "axon_start_nrt_profile"):
        return
    lib.axon_start_nrt_profile.argtypes = [
        ctypes.POINTER(ctypes.c_int64),
        ctypes.c_size_t,
    ]
    lib.axon_start_nrt_profile.restype = ctypes.c_int64
    lib.axon_stop_nrt_profile.argtypes = [ctypes.c_char_p]
    lib.axon_stop_nrt_profile.restype = ctypes.c_int64

    @contextlib.contextmanager
    def _hook(output_dir, device_ids):
        import jax

        jax.devices()
        if device_ids:
            ids = (ctypes.c_int64 * len(device_ids))(*device_ids)
            rc = lib.axon_start_nrt_profile(ids, len(device_ids))
        else:
            rc = lib.axon_start_nrt_profile(None, 0)
        if rc != 0:
            raise RuntimeError(f"axon_start_nrt_profile rc={rc}")
        try:
            yield
        finally:
            n = lib.axon_stop_nrt_profile(str(output_dir).encode())
            print(f"ntff profile: {n} file(s) -> {output_dir}", file=sys.stderr)

    mod = types.ModuleType("antenv.axon_hooks")
    mod.get_axon_ntff_profile_hook = lambda: _hook
    mod.set_axon_ntff_profile_hook = lambda h: None
    sys.modules["antenv.axon_hooks"] = mod
    import antenv

    antenv.axon_hooks = mod


# ------------------------------------------------------------ host preprocessing
def _chunk_part_to_local(c, p):
    """local slot for (G-phase chunk c, partition p) within a core (vectorized)."""
    c = np.asarray(c)
    g = c // 8
    j = c - g * 8
    gbase = np.asarray(GBASE)[g]
    gj = np.asarray(GJ)[g]
    return gbase + p * gj + j


def _local_to_chunk_part(l):
    """(chunk, partition) for a local slot (vectorized)."""
    g = np.minimum(l // 1024, 2)
    rem = l - g * 1024
    jg = np.where(g < 2, 8, 4)
    p = rem // jg
    j = rem - p * jg
    return g * 8 + j, p


def _wrap_idx16(flat):
    """Wrap a flat int16 index vector into the dma_gather SBUF image:
    index i -> partition i%16, column i//16, replicated 8x down partitions."""
    assert flat.size % 16 == 0
    v = flat.reshape(-1, 16).T.astype(np.int16)
    return np.tile(v, (8, 1))


def preprocess(x, edge_index, W1, att_src1, att_dst1, b1, W2, att_src2, att_dst2, b2):
    src = np.asarray(edge_index[0], dtype=np.int64)
    dst = np.asarray(edge_index[1], dtype=np.int64)
    loops = np.arange(N, dtype=np.int64)
    src = np.concatenate([src, loops])
    dst = np.concatenate([dst, loops])

    deg = np.bincount(dst, minlength=N)
    order = np.argsort(-deg, kind="stable")  # rank -> node
    ranks = np.arange(N)
    # rank r -> core r%8, chunk (r//8)//128, partition (r//8)%128
    core = ranks % NCORES
    cc = (ranks // NCORES) // 128
    pp = (ranks // NCORES) % 128
    node2slot = np.empty(N, dtype=np.int64)
    node2slot[order] = core * PN + _chunk_part_to_local(cc, pp)
    # rank -> output row (out_d is chunk-major: c*128+p), per core
    node2outrow = np.empty(N, dtype=np.int64)
    node2outrow[order] = core * PN + cc * 128 + pp

    es = node2slot[src]
    ed = node2slot[dst]

    # per-destination edge schedule
    o2 = np.argsort(ed, kind="stable")
    s_src = es[o2]
    s_dst = ed[o2]
    starts = np.searchsorted(s_dst, np.arange(NSLOT))
    kwithin = np.arange(s_dst.size) - starts[s_dst]

    deg_slot = np.bincount(ed, minlength=NSLOT)
    dloc = deg_slot.reshape(NCORES, PN)
    lc, lp = _local_to_chunk_part(np.arange(PN))
    dmat = np.zeros((NCORES, CH, 128), dtype=np.int64)
    dmat[:, lc, lp] = dloc
    dsched = dmat.max(axis=(0, 2))
    dsched = np.maximum(dsched, 1).astype(np.int64)
    choff = np.concatenate([[0], np.cumsum(dsched)])
    TOTD = int(choff[-1])

    # per-core [TOTD, 128] neighbor tables filled with PADROW
    arr = np.full((NCORES, TOTD, 128), PADROW, dtype=np.int64)
    e_core = s_dst // PN
    e_loc = s_dst % PN
    e_ch, e_p = _local_to_chunk_part(e_loc)
    arr[e_core, choff[e_ch] + kwithin, e_p] = s_src
    assert arr.max() <= 32767

    idx_imgs = [_wrap_idx16(arr[k].reshape(-1).astype(np.int16)) for k in range(NCORES)]
    own_imgs = []
    ownrows = np.empty(CH * 128, dtype=np.int64)
    for c in range(CH):
        ownrows[c * 128 : (c + 1) * 128] = _chunk_part_to_local(c, np.arange(128))
    for k in range(NCORES):
        own_imgs.append(_wrap_idx16((k * PN + ownrows).astype(np.int16)))

    # permuted node features (by slot), transposed, padded
    xT = np.zeros((IN, NSLOT), dtype=np.float32)
    xT[:, node2slot] = np.asarray(x, dtype=np.float32).T

    # W extensions: columns reordered (c*H + h); attention projections folded in
    W1r = np.asarray(W1, np.float32).reshape(IN, H, HF)
    w1p = W1r.transpose(0, 2, 1).reshape(IN, H * HF)
    a1s = np.einsum("ihc,hc->ih", W1r, np.asarray(att_src1, np.float32))
    a1d = np.einsum("ihc,hc->ih", W1r, np.asarray(att_dst1, np.float32))
    w1e = np.concatenate([w1p, a1s, a1d], axis=1).astype(np.float32)  # [IN, 528]

    W2r = np.asarray(W2, np.float32).reshape(HF, H, OUT)
    w2p = W2r.transpose(0, 2, 1).reshape(HF, H * OUT)
    a2s = np.einsum("ihc,hc->ih", W2r, np.asarray(att_src2, np.float32))
    a2d = np.einsum("ihc,hc->ih", W2r, np.asarray(att_dst2, np.float32))
    w2e = np.concatenate([w2p, a2s, a2d], axis=1).astype(np.float32)  # [HF, 272]

    import ml_dtypes

    b1b = np.tile(np.asarray(b1, np.float32)[None, :], (128, 1))
    b2b = np.tile(np.asarray(b2, np.float32)[None, :], (128, 1))
    identf = np.eye(128, dtype=np.float32)
    identbf = np.eye(128, dtype=ml_dtypes.bfloat16)

    # ximg per parity: A1 lhsT tiles for core-blocks [par, par+2, par+4, par+6]
    # laid out per A-chunk a = i*CH + c:
    # ximg[p, a*256 + kk*128 + n] = x-feat (kk*128+p) of dst-node (chunk c, part n)
    xT_b = xT.astype(ml_dtypes.bfloat16)
    im = np.zeros((128, 160 * 256), dtype=ml_dtypes.bfloat16)
    a = 0
    for kb in range(NCORES):
        for c in range(CH):
            slots = kb * PN + ownrows[c * 128 : (c + 1) * 128]
            blk = xT_b[:, slots]  # [256, 128]
            im[:, a * 256 : a * 256 + 128] = blk[0:128]
            im[:, a * 256 + 128 : a * 256 + 256] = blk[128:256]
            a += 1
    ximg = im

    shared = {
        "w1e": w1e.astype(ml_dtypes.bfloat16),
        "w2e": w2e.astype(ml_dtypes.bfloat16),
        "b1b": b1b,
        "b2b": b2b,
        "identf": identf,
        "identbf": identbf,
    }
    in_maps = []
    for k in range(NCORES):
        m = dict(shared)
        m["ximg"] = ximg
        m["idxs"] = idx_imgs[k]
        m["ownidx"] = own_imgs[k]
        in_maps.append(m)

    meta = {
        "dsched": tuple(int(d) for d in dsched),
        "choff": tuple(int(c) for c in choff),
        "node2outrow": node2outrow,
    }
    return in_maps, meta


# ------------------------------------------------------------------ the program
def build_program(meta, ncores=NCORES, enable_asserts=False, debug=False):
    dsched = meta["dsched"]
    choff = meta["choff"]
    TOTD = choff[-1]

    nc = bacc.Bacc(
        "TRN2",
        target_bir_lowering=False,
        debug=debug,
        enable_asserts=enable_asserts,
        num_devices=ncores,
        num_swdge_queues=4,
    )

    # ---- I/O
    ximg_d = nc.dram_tensor("ximg", [128, 160 * 256], BF16, kind="ExternalInput")
    w1e = nc.dram_tensor("w1e", [IN, D1 + 16], BF16, kind="ExternalInput")
    w2e = nc.dram_tensor("w2e", [HF, D2 + 16], BF16, kind="ExternalInput")
    b1b_d = nc.dram_tensor("b1b", [128, HF], F32, kind="ExternalInput")
    b2b_d = nc.dram_tensor("b2b", [128, OUT], F32, kind="ExternalInput")
    identf_d = nc.dram_tensor("identf", [128, 128], F32, kind="ExternalInput")
    identbf_d = nc.dram_tensor("identbf", [128, 128], BF16, kind="ExternalInput")
    idxs_d = nc.dram_tensor("idxs", [128, TOTD * 8], I16, kind="ExternalInput")
    ownidx_d = nc.dram_tensor("ownidx", [128, PN // 16], I16, kind="ExternalInput")
    out_d = nc.dram_tensor("out", [PN, OUT], F32, kind="ExternalOutput")

    # ---- pair-shared tables (one physical copy per 2-core HBM domain)
    h1_tbl = nc.dram_tensor("h1_tbl", [NROWS, RW1], BF16)
    h2_tbl = nc.dram_tensor("h2_tbl", [NROWS, RW2], FP8)
    h1t_loc = nc.dram_tensor("h1t_loc", [HF, PN], BF16)
    h1t_all = nc.dram_tensor("h1t_all", [ncores, HF, PN], BF16, addr_space="Shared")

    _swprev = [None]
    _swq = [0]

    def _gather(out_ap, in_ap, idxs_ap, nidx, elem):
        q = _swq[0] % 4
        _swq[0] += 1
        inst = nc.gpsimd.dma_gather(
            out_ap, in_ap, idxs_ap, nidx, nidx, elem, queue_num=q
        )
        if _swprev[0] is not None:
            bass._add_dep_helper(
                inst.ins, _swprev[0].ins, sync=False, reason="swdge order"
            )
        _swprev[0] = inst
        return inst

    with tile.TileContext(nc) as tc:

        with contextlib.ExitStack() as big:
            cpool = big.enter_context(tc.tile_pool(name="consts", bufs=1))
            w1_sb = cpool.tile([128, 2, D1 + 16], BF16)
            nc.sync.dma_start(w1_sb[:], w1e[:, :].rearrange("(k p) c -> p k c", p=128))
            w2_sb = cpool.tile([HF, D2 + 16], BF16)
            nc.sync.dma_start(w2_sb[:], w2e[:, :])
            b1_sb = cpool.tile([128, HF], F32)
            nc.sync.dma_start(b1_sb[:], b1b_d[:, :])
            b2_sb = cpool.tile([128, OUT], F32)
            nc.sync.dma_start(b2_sb[:], b2b_d[:, :])
            idf_sb = cpool.tile([128, 128], F32)
            nc.sync.dma_start(idf_sb[:], identf_d[:, :])
            idb_sb = cpool.tile([128, 128], BF16)
            nc.sync.dma_start(idb_sb[:], identbf_d[:, :])
            idx_sb = cpool.tile([128, TOTD * 8], I16)
            nc.sync.dma_start(idx_sb[:], idxs_d[:, :])
            own_sb = cpool.tile([128, PN // 16], I16)
            nc.sync.dma_start(own_sb[:], ownidx_d[:, :])
            neg_sb = cpool.tile([1, 8], BF16)
            nc.vector.memset(neg_sb[:], NEG_BIG)
            zro_sb = cpool.tile([1, D1], BF16)
            nc.vector.memset(zro_sb[:], 0.0)
            zro8_sb = cpool.tile([1, D2], FP8)
            nc.vector.memset(zro8_sb[:], 0.0)
            eps_sb = cpool.tile([128, 1], F32)
            nc.vector.memset(eps_sb[:], 1e-30)
            h1t_sb = cpool.tile([HF, PN], BF16)

            # ================ phase A helper: project all 8 core-blocks;
            # rows blk*PN + GBASE[g] + p*jg + j, written contiguously.
            def phase_A(lhsT_of, w_of, kparts, dcols, rw, tbl, tag, fp8=False):
                fuse = dcols + 16 <= 512
                last_write = [None]
                with contextlib.ExitStack() as st:
                    lp = st.enter_context(tc.tile_pool(name=f"a{tag}l", bufs=3))
                    pp = st.enter_context(
                        tc.tile_pool(name=f"a{tag}p", bufs=3, space="PSUM")
                    )
                    sp = st.enter_context(tc.tile_pool(name=f"a{tag}s", bufs=4))
                    a = 0
                    for blk in range(ncores):
                        for g in range(3):
                            jg = GJ[g]
                            hst = sp.tile(
                                [128, 8, rw], FP8 if fp8 else BF16, tag=f"hst{tag}"
                            )
                            for j in range(jg):
                                lhsT = lhsT_of(lp, a)
                                a += 1
                                if fuse:
                                    ps = pp.tile([128, dcols + 16], F32, tag=f"ps{tag}")
                                    for kk in range(kparts):
                                        nc.tensor.matmul(
                                            ps[:],
                                            lhsT(kk),
                                            w_of(kk)[:, 0 : dcols + 16],
                                            start=(kk == 0),
                                            stop=(kk == kparts - 1),
                                        )
                                    if fp8:
                                        if j % 2 == 0:
                                            nc.scalar.copy(
                                                hst[:, j, 0:dcols], ps[:, 0:dcols]
                                            )
                                        else:
                                            nc.vector.tensor_copy(
                                                hst[:, j, 0:dcols], ps[:, 0:dcols]
                                            )
                                        nc.vector.tensor_copy(
                                            hst[:, j, dcols : dcols + 32].bitcast(BF16),
                                            ps[:, dcols : dcols + 16],
                                        )
                                    elif j % 2 == 0:
                                        nc.scalar.copy(hst[:, j, 0 : dcols + 16], ps[:])
                                    else:
                                        nc.vector.tensor_copy(
                                            hst[:, j, 0 : dcols + 16], ps[:]
                                        )
                                else:
                                    ps = pp.tile([128, dcols], F32, tag=f"ps{tag}")
                                    ps2 = pp.tile([128, 16], F32, tag=f"ps2{tag}")
                                    for kk in range(kparts):
                                        nc.tensor.matmul(
                                            ps[:],
                                            lhsT(kk),
                                            w_of(kk)[:, 0:dcols],
                                            start=(kk == 0),
                                            stop=(kk == kparts - 1),
                                        )
                                        nc.tensor.matmul(
                                            ps2[:],
                                            lhsT(kk),
                                            w_of(kk)[:, dcols : dcols + 16],
                                            start=(kk == 0),
                                            stop=(kk == kparts - 1),
                                        )
                                    if j % 2 == 0:
                                        nc.scalar.copy(hst[:, j, 0:dcols], ps[:])
                                    else:
                                        nc.vector.tensor_copy(hst[:, j, 0:dcols], ps[:])
                                    nc.vector.tensor_copy(
                                        hst[:, j, dcols : dcols + 16], ps2[:]
                                    )
                            row0 = blk * PN + GBASE[g]
                            w = nc.sync.dma_start(
                                tbl[row0 : row0 + jg * 128, :].rearrange(
                                    "(p j) c -> p j c", p=128
                                ),
                                hst[:, 0:jg, :],
                            )
                            last_write[0] = w
                return last_write[0]

            # ---------------- A1 (x tiles loaded one core-block at a time)
            _xblk = [None, -1]

            def lhsT_of_A1(lp, a):
                blk = a // CH
                if _xblk[1] != blk:
                    xt = lp.tile([128, CH * 256], BF16, tag="xt")
                    nc.sync.dma_start(
                        xt[:], ximg_d[:, blk * CH * 256 : (blk + 1) * CH * 256]
                    )
                    _xblk[0] = xt
                    _xblk[1] = blk
                base = (a % CH) * 256
                xt = _xblk[0]
                return lambda kk: xt[:, base + kk * 128 : base + (kk + 1) * 128]

            lastw1 = phase_A(
                lhsT_of_A1, lambda kk: w1_sb[:, kk, :], 2, D1, RW1, h1_tbl, "1"
            )

            # PADROW fixups (both cores of a pair write the same values)
            f1 = nc.sync.dma_start(h1_tbl[PADROW : PADROW + 1, 0:D1], zro_sb[:, 0:D1])
            f2 = nc.sync.dma_start(
                h1_tbl[PADROW : PADROW + 1, D1 : D1 + 8], neg_sb[:, 0:8]
            )


            # ================ gather/aggregate phase
            def phase_G(tbl, dcols, rw, chans, b_sb, relu, sink, tag, hbufs,
                        fp8=False, after_chunk=None):
                with contextlib.ExitStack() as st:
                    gp = st.enter_context(tc.tile_pool(name=f"g{tag}g", bufs=1))
                    hp = st.enter_context(tc.tile_pool(name=f"g{tag}h", bufs=hbufs))
                    sp = st.enter_context(tc.tile_pool(name=f"g{tag}s", bufs=3))
                    mp = st.enter_context(tc.tile_pool(name=f"g{tag}m", bufs=4))
                    xp = st.enter_context(tc.tile_pool(name=f"g{tag}x", bufs=2))
                    pp = st.enter_context(
                        tc.tile_pool(name=f"g{tag}p", bufs=2, space="PSUM")
                    )
                    def gather(out_ap, idxs_ap, nidx, need_bar):
                        _gather(out_ap, tbl[:, :], idxs_ap, nidx, rw)

                    # own rows (adst at dcols+8:dcols+16): self-written in
                    # phase A (parity blocks), so no barrier needed.
                    atown = gp.tile(
                        [128, CH, rw], FP8 if fp8 else BF16, tag=f"at{tag}"
                    )
                    off = 0
                    while off < PN:
                        nn = min(1024, PN - off)
                        gather(
                            atown[:, off // 128 : (off + nn) // 128, :],
                            own_sb[:, off // 16 : (off + nn) // 16],
                            nn,
                            False,
                        )
                        off += nn

                    for c in range(CH):
                        D = dsched[c]
                        col0 = choff[c] * 8
                        ex = mp.tile([128, 64, H], BF16, tag=f"ex{tag}")
                        ps = pp.tile([128, dcols], F32, tag=f"ps{tag}")
                        if fp8:
                            adst_c = atown[:, c, dcols + 16 : dcols + 32].bitcast(BF16)
                        else:
                            adst_c = atown[:, c, dcols + 8 : dcols + 16]
                        first = True
                        for d0 in range(0, D, 16):
                            dn = min(16, D - d0)
                            hg = hp.tile(
                                [128, 16, rw], FP8 if fp8 else BF16, tag=f"hg{tag}"
                            )
                            for dd in range(0, dn, 8):
                                ddn = min(8, dn - dd)
                                gather(
                                    hg[:, dd : dd + ddn, :],
                                    idx_sb[
                                        :,
                                        col0 + (d0 + dd) * 8 : col0 + (d0 + dd + ddn) * 8,
                                    ],
                                    ddn * 128,
                                    True,
                                )
                            # logits for the whole 32-slice group
                            epre = mp.tile([128, 16, H], F32, tag=f"ep{tag}")
                            adst_b = adst_c.unsqueeze(1).broadcast_to([128, dn, H])
                            asrc_v = (
                                hg[:, 0:dn, dcols : dcols + 16].bitcast(BF16)
                                if fp8
                                else hg[:, 0:dn, dcols : dcols + 8]
                            )
                            nc.vector.tensor_tensor(
                                epre[:, 0:dn, :], asrc_v, adst_b, OP.add
                            )
                            e = mp.tile([128, 16, H], F32, tag=f"e{tag}")
                            nc.vector.scalar_tensor_tensor(
                                e[:, 0:dn, :],
                                epre[:, 0:dn, :],
                                NEG_SLOPE,
                                epre[:, 0:dn, :],
                                OP.mult,
                                OP.max,
                            )
                            nc.scalar.activation(
                                ex[:, d0 : d0 + dn, :], e[:, 0:dn, :], AF.Exp
                            )
                            for dd in range(0, dn, 8):
                                ddn = min(8, dn - dd)
                                sg = sp.tile([128, 8, dcols], BF16, tag=f"sg{tag}")
                                hg_v = hg[:, dd : dd + ddn, 0:dcols].rearrange(
                                    "p d (c h) -> p d c h", h=H
                                )
                                sg_v = sg[:, 0:ddn, :].rearrange(
                                    "p d (c h) -> p d c h", h=H
                                )
                                ex_v = (
                                    ex[:, d0 + dd : d0 + dd + ddn, :]
                                    .unsqueeze(2)
                                    .broadcast_to([128, ddn, chans, H])
                                )
                                nc.vector.tensor_tensor(sg_v, hg_v, ex_v, OP.mult)
                                for j in range(ddn):
                                    nc.tensor.matmul(
                                        ps[:],
                                        idb_sb[:],
                                        sg[:, j, :],
                                        start=first,
                                        stop=(d0 + dd + ddn >= D and j == ddn - 1),
                                    )
                                    first = False
                        den = mp.tile([128, H], F32, tag=f"den{tag}")
                        nc.vector.reduce_sum(
                            den[:],
                            ex[:, 0:D, :].transpose([0, 2, 1]),
                            axis=mybir.AxisListType.X,
                        )
                        nc.scalar.activation(
                            den[:], den[:], AF.Identity, bias=eps_sb[:, 0:1]
                        )
                        rden = mp.tile([128, H], F32, tag=f"rd{tag}")
                        nc.vector.reciprocal(rden[:], den[:])
                        acc = mp.tile([128, dcols], F32, tag=f"ac{tag}")
                        nc.scalar.copy(acc[:], ps[:])
                        t1 = mp.tile([128, dcols], F32, tag=f"t1{tag}")
                        rden_b = rden[:].unsqueeze(1).broadcast_to([128, chans, H])
                        nc.vector.tensor_tensor(
                            t1[:].rearrange("p (c h) -> p c h", h=H),
                            acc[:].rearrange("p (c h) -> p c h", h=H),
                            rden_b,
                            OP.mult,
                        )
                        hsum = mp.tile([128, chans], F32, tag=f"hs{tag}")
                        nc.vector.reduce_sum(
                            hsum[:],
                            t1[:].rearrange("p (c h) -> p c h", h=H),
                            axis=mybir.AxisListType.X,
                        )
                        res = mp.tile([128, chans], F32, tag=f"rs{tag}")
                        nc.vector.scalar_tensor_tensor(
                            res[:], hsum[:], 1.0 / H, b_sb[:], OP.mult, OP.add
                        )
                        if relu:
                            resf = mp.tile([128, chans], F32, tag=f"rf{tag}")
                            nc.scalar.activation(resf[:], res[:], AF.Relu)
                        else:
                            resf = res
                        sink(c, resf, xp, pp)
                        if after_chunk is not None:
                            after_chunk(c)

            # ---------------- G1 -> h1t_sb, AG + A2 overlapped in halves
            def sink1(c, resf, xp, pp):
                tps = pp.tile([HF, 128], F32, tag="tps")
                nc.tensor.transpose(tps[:], resf[:], idf_sb[:])
                nc.scalar.copy(h1t_sb[:, c * 128 : (c + 1) * 128], tps[:])

            phase_G(h1_tbl, D1, RW1, HF, b1_sb, True, sink1, "1", 5)
            nc.sync.dma_start(h1t_loc[:, :], h1t_sb[:])
            nc.gpsimd.collective_compute(
                "AllGather",
                OP.bypass,
                replica_groups=[list(range(ncores))],
                ins=[h1t_loc[:, :]],
                outs=[h1t_all[:, :, :]],
            )

            # ---------------- A2: project all nodes from the gathered h1T
            with tc.tile_pool(name="h1tp", bufs=1) as h1tp:
                h1tf = h1tp.tile([HF, NSLOT], BF16)
                nc.sync.dma_start(
                    h1tf[:].rearrange("p (j c) -> p j c", c=PN),
                    h1t_all[:, :, :].transpose([1, 0, 2]),
                )

                def lhsT_of_A2(lp, a):
                    return lambda kk: h1tf[:, a * 128 : (a + 1) * 128]

                lastw2 = phase_A(lhsT_of_A2, lambda kk: w2_sb, 1, D2, RW2, h2_tbl, "2", fp8=True)

            f3 = nc.sync.dma_start(
                h2_tbl[PADROW : PADROW + 1, 0:D2], zro8_sb[:, 0:D2]
            )
            f4 = nc.sync.dma_start(
                h2_tbl[PADROW : PADROW + 1, D2 : D2 + 16].bitcast(BF16),
                neg_sb[:, 0:8],
            )


            # ---------------- G2 -> out
            ostage = [None]

            def sink2(c, resf, xp, pp):
                r = c % 4
                if r == 0:
                    ost = xp.tile([128, 4, OUT], F32, tag="ost")
                    ostage[0] = ost
                nc.vector.tensor_copy(ostage[0][:, r, :], resf[:])
                if r == 3 or c == CH - 1:
                    c0 = c - r
                    nc.sync.dma_start(
                        out_d[c0 * 128 : (c + 1) * 128, :].rearrange(
                            "(r p) c -> p r c", p=128
                        ),
                        ostage[0][:, 0 : r + 1, :],
                    )

            phase_G(h2_tbl, D2, RW2, OUT, b2_sb, False, sink2, "2", 12, fp8=True)

    nc.compile()
    return nc


# ------------------------------------------------------------------ entry point
_CACHE = {}


def _get_program(meta):
    key = (meta["dsched"],)
    if key not in _CACHE:
        _CACHE[key] = build_program(meta)
    return _CACHE[key]


def kernel(x, edge_index, W1, att_src1, att_dst1, b1, W2, att_src2, att_dst2, b2,
           trace=False):
    global LAST_EXEC_TIME_NS
    _install_ntff_hook()
    in_maps, meta = preprocess(
        x, edge_index, W1, att_src1, att_dst1, b1, W2, att_src2, att_dst2, b2
    )
    nc = _get_program(meta)
    res = run_bass_kernel_spmd(nc, in_maps, list(range(NCORES)), trace=trace)
    LAST_EXEC_TIME_NS = res.exec_time_ns
    outs = np.concatenate([res.results[k]["out"] for k in range(NCORES)], axis=0)
    return outs[meta["node2outrow"]].astype(np.float32)
